# revision 23
# baseline (speedup 1.0000x reference)
"""Dense-MoE (top-2 of 8 experts) TRN2 kernel v8: expert-parallel, bf16 matmuls.

Host side: softmax + top-2 routing, per-expert token gather (padded to the max
expert load), weight re-layout + bf16 conversion. Device side (per core = one
expert), all matmul operands bf16, PSUM accumulation f32:
    phase A:  h[f, c] = silu(gw @ x) * (uw @ x)      [f-major, bf16 in SBUF]
    phase B:  outT[d, c] = sum_f dwT[f, d] * h[f, c]  [tokens on the free dim]
The routing weight and the scatter-add back to [T, D] happen on the host
(out[t] += w_t * outT[:, c].T), so the device kernel needs no tw input.

vs v4 (193.0us baseline): the last d-tile of phase B runs its chunks
sequentially (accumulation groups close at staggered intervals) AND splits
the final chunk in two, so the kernel-ending PSUM drain + out DMA are
half-sized and overlap the preceding sub-chunks' matmuls.

Profile anatomy at ~193.7us (cool device): ~5.6us DMA-bound ramp (x 2.2MB +
ft0/ft1 weights 1.5MB at ~370 GB/s aggregate; the PE p-state runs at half
speed until ~3us after the last ramp stall), ~177us matmul span (1152
matmuls, 98% of the 2.37 GHz sustained issue rate - the bf16 roofline at
cap~1058 is 171us), ~2.7us final drain + DMA latency, and ~8us of
NEFF semaphore-teardown (walrus zeroes all 253 sems per-engine) that is
measured but not kernel-addressable.

Negative results (measured, don't retry):
  - fp8 DoubleRow is exactly 2x bf16 on HW (216ns per 512-col 2-ktile
    matmul), but pure-fp8 absmax error is 7e-2 vs the 2e-2 budget and
    hi/lo-compensated fp8 (3 products per 2 ktiles) is 1.5x bf16 time.
  - PE p-state prewarm via dummy matmuls, chunk-major x layout + ci-outer
    phase A, finer x slicing, and full-tile/queue-rebalanced startup
    weights all measured neutral-to-worse (194.7-196.4us): any >=1us PE
    stall resets the clock to half speed for ~3us, and ft0 is inherently
    DMA-bound, so earlier starts just move the stalls around.
  - Using the GpSimd/Pool engine in the body coincided with ~1.2x-slow
    runs; later shown to be a transient device DVFS state (~1.95 GHz) that
    any back-to-back run can hit and idle time clears - but keep gpsimd
    out of the body anyway.
  - fp16 is ~25% slower than bf16 on the PE; GpSimd-issued DMAs land on a
    software-dynamic DGE queue with ~1.6us drains (both from v4's notes).
"""
import sys

sys.path.insert(0, "/opt/trn_rl_repo")

import ml_dtypes
import numpy as np

import concourse.bass as bass
from concourse import bacc
import concourse.mybir as mybir
import concourse.tile as tile
from concourse.bass_utils import run_bass_kernel_spmd
from concourse.bass import ds

T, D, F, E, TOPK = 4096, 1024, 2048, 8, 2
P = 128
N_CORES = 8

F32 = mybir.dt.float32
BF16 = mybir.dt.bfloat16
BF = ml_dtypes.bfloat16


def _chunks(cap, n):
    """Split [0, cap) into n even-sized chunks (sizes even, ~balanced)."""
    base = (cap // n) & ~1
    sizes = [base] * n
    rem = cap - base * n
    i = 0
    while rem > 0:
        sizes[i % n] += 2
        rem -= 2
        i += 1
    out = []
    c0 = 0
    for cs in sizes:
        out.append((c0, cs))
        c0 += cs
    return out


def _build(cap):
    n_ch = -(-cap // 512)  # token chunks (<=512 fp32 psum free dim)
    chunks = _chunks(cap, n_ch)
    nch = len(chunks)
    assert nch <= 4, f"max expert load {cap} needs {nch} token chunks > 4"

    # x is packed with ft0/ft1's weight slices: per d-slice row, the cap x
    # columns are followed by gw0/uw0/gw1/uw1's 128 fi-columns each (the
    # stationary tiles share the d-inner partition dim with x), so the
    # monopoly-queue descriptor stream delivers the ramp's critical weights
    # at full bandwidth instead of crawling on the starved sync queue.
    W = cap + 4 * P
    nc = bacc.Bacc(None, target_bir_lowering=False)
    x_d = nc.declare_dram_parameter("x", [P, D // P, W], BF16, isOutput=False)
    gw_d = nc.declare_dram_parameter("gw", [P, F // P, D // P, P], BF16, isOutput=False)
    uw_d = nc.declare_dram_parameter("uw", [P, F // P, D // P, P], BF16, isOutput=False)
    dw_d = nc.declare_dram_parameter("dw", [P, F // P, D], BF16, isOutput=False)
    out_d = nc.declare_dram_parameter("out", [P, D // P, cap], BF16, isOutput=True)

    with tile.TileContext(nc) as tc:
        with (
            tc.tile_pool(name="deep", bufs=1) as deep,
            tc.tile_pool(name="wts", bufs=3) as wts,
            tc.tile_pool(name="stage", bufs=2) as stage,
            tc.tile_pool(name="ps", bufs=1, space="PSUM") as ps,
        ):
            wt_tiles = {}

            def load_ft(ft):
                gw_t = wts.tile([P, D // P, P], BF16, tag="gw", name="gw_t")
                nc.sync.dma_start(gw_t[:], gw_d[:, ft])
                uw_t = wts.tile([P, D // P, P], BF16, tag="uw", name="uw_t")
                nc.sync.dma_start(uw_t[:], uw_d[:, ft])
                wt_tiles[ft] = (gw_t, uw_t)

            # The packed x stream rides the scalar queue alone (3.1KB lines
            # monopolize the DMA arbiter, which tracks head-of-queue line
            # size; the ramp's critical path is exactly this stream). The
            # first two d-slices go as single-slice descriptors so the
            # first gate chain starts ~1us sooner; sync carries only ft2+
            # full-tile weight loads, which crawl during x flight and are
            # not needed before ~17us.
            x_t = deep.tile([P, D // P, W], BF16, tag="x")
            nc.scalar.dma_start(x_t[:, ds(0, 1)], x_d[:, ds(0, 1)])
            nc.scalar.dma_start(x_t[:, ds(1, 1)], x_d[:, ds(1, 1)])
            for dt_ in range(2, D // P, 2):
                nc.scalar.dma_start(x_t[:, ds(dt_, 2)], x_d[:, ds(dt_, 2)])

            # ft2/ft3 load during x's flight, so they MUST be 512B crawl
            # slices (full-tile 2KB lines grab an early arbiter share and
            # delay x by several us -- measured). ft4+ load full-tile after
            # the crawl drains, when x is done and sync has full bandwidth.
            def load_ft_sliced(ft):
                gw_t = wts.tile([P, D // P, P], BF16, tag="gw", name="gw_t")
                uw_t = wts.tile([P, D // P, P], BF16, tag="uw", name="uw_t")
                for dt_ in range(0, D // P, 2):
                    nc.sync.dma_start(gw_t[:, ds(dt_, 2)], gw_d[:, ft, ds(dt_, 2)])
                    nc.sync.dma_start(uw_t[:, ds(dt_, 2)], uw_d[:, ft, ds(dt_, 2)])
                wt_tiles[ft] = (gw_t, uw_t)

            load_ft_sliced(2)
            load_ft_sliced(3)

            h_t = deep.tile([P, F // P, cap], BF16, tag="h")
            dw_t = deep.tile([P, F // P, D], BF16, tag="dw")

            for ft in range(F // P):
                if ft == 8:
                    for fo in range(0, F // P, 4):
                        nc.sync.dma_start(dw_t[:, ds(fo, 4)], dw_d[:, ds(fo, 4)])
                if 2 <= ft + 1 < F // P and (ft + 1) not in wt_tiles:
                    load_ft(ft + 1)
                if ft < 2:
                    # ft0/ft1 stationary tiles live in the packed x tile.
                    def gsl(dt_, ft=ft):
                        return x_t[:, dt_, ds(cap + ft * 2 * P, P)]

                    def usl(dt_, ft=ft):
                        return x_t[:, dt_, ds(cap + ft * 2 * P + P, P)]
                else:
                    gw_t, uw_t = wt_tiles.pop(ft)

                    def gsl(dt_, t=gw_t):
                        return t[:, dt_]

                    def usl(dt_, t=uw_t):
                        return t[:, dt_]
                pgs = [ps.tile([P, 512], F32, tag=f"pg{ci}", name=f"pg{ci}") for ci in range(nch)]
                pus = [ps.tile([P, 512], F32, tag=f"pu{ci}", name=f"pu{ci}") for ci in range(nch)]
                for dt_ in range(D // P):
                    for ci, (c0, cs) in enumerate(chunks):
                        nc.tensor.matmul(
                            pgs[ci][:, :cs], gsl(dt_), x_t[:, dt_, ds(c0, cs)],
                            start=(dt_ == 0), stop=(dt_ == D // P - 1),
                        )
                    for ci, (c0, cs) in enumerate(chunks):
                        nc.tensor.matmul(
                            pus[ci][:, :cs], usl(dt_), x_t[:, dt_, ds(c0, cs)],
                            start=(dt_ == 0), stop=(dt_ == D // P - 1),
                        )
                sgs = []
                for ci, (c0, cs) in enumerate(chunks):
                    sg = stage.tile([P, 512], BF16, tag=f"sg{ci}", name=f"sg{ci}")
                    nc.scalar.activation(sg[:, :cs], pgs[ci][:, :cs],
                                         mybir.ActivationFunctionType.Silu)
                    sgs.append(sg)
                for ci, (c0, cs) in enumerate(chunks):
                    nc.vector.tensor_tensor(
                        h_t[:, ft, ds(c0, cs)], sgs[ci][:, :cs], pus[ci][:, :cs],
                        mybir.AluOpType.mult,
                    )

            for dt_ in range(D // P):
                grp, other = ("pg", "pu") if dt_ % 2 == 0 else ("pu", "pg")
                last = dt_ == D // P - 1
                if last:
                    # Sequential sub-chunks, with the final chunk split so the
                    # kernel-ending PSUM drain + out DMA are small: each
                    # accumulation group closes at a staggered interval and
                    # its drain copy + per-slice DMA overlap the remaining
                    # sub-chunks' matmuls.
                    lc0, lcs = chunks[-1]
                    lh = ((lcs // 2) + 1) & ~1
                    sub = list(chunks[:-1]) + [(lc0, lh), (lc0 + lh, lcs - lh)]
                    tags = [f"{grp}{ci}" for ci in range(nch)] + [f"{other}0"]
                    osb = stage.tile([P, cap], BF16, tag="osb", name="osb")
                    for ci, (c0, cs) in enumerate(sub):
                        po = ps.tile([P, 512], F32, tag=tags[ci], name=f"po{ci}")
                        for fo in range(F // P):
                            nc.tensor.matmul(
                                po[:, :cs], dw_t[:, fo, ds(dt_ * P, P)],
                                h_t[:, fo, ds(c0, cs)],
                                start=(fo == 0), stop=(fo == F // P - 1),
                            )
                        if ci % 2 == 0:
                            nc.scalar.activation(osb[:, ds(c0, cs)], po[:, :cs],
                                                 mybir.ActivationFunctionType.Copy)
                        else:
                            nc.vector.tensor_scalar_mul(osb[:, ds(c0, cs)], po[:, :cs], 1.0)
                        nc.sync.dma_start(out_d[:, dt_, ds(c0, cs)], osb[:, ds(c0, cs)])
                else:
                    pos = [ps.tile([P, 512], F32, tag=f"{grp}{ci}", name=f"po{ci}") for ci in range(nch)]
                    for fo in range(F // P):
                        for ci, (c0, cs) in enumerate(chunks):
                            nc.tensor.matmul(
                                pos[ci][:, :cs], dw_t[:, fo, ds(dt_ * P, P)],
                                h_t[:, fo, ds(c0, cs)],
                                start=(fo == 0), stop=(fo == F // P - 1),
                            )
                    osb = stage.tile([P, cap], BF16, tag="osb", name="osb")
                    for ci, (c0, cs) in enumerate(chunks):
                        if ci % 2 == 0:
                            nc.scalar.activation(osb[:, ds(c0, cs)], pos[ci][:, :cs],
                                                 mybir.ActivationFunctionType.Copy)
                        else:
                            nc.vector.tensor_scalar_mul(osb[:, ds(c0, cs)], pos[ci][:, :cs], 1.0)
                    nc.sync.dma_start(out_d[:, dt_], osb[:])
    nc.finalize()
    return nc


def _route(gating_output):
    g = gating_output.astype(np.float32)
    m = g.max(axis=-1, keepdims=True)
    e = np.exp(g - m)
    probs = e / e.sum(axis=-1, keepdims=True)
    ids = np.argsort(-probs, axis=-1, kind="stable")[:, :TOPK]
    w = np.take_along_axis(probs, ids, axis=-1)
    w = w / w.sum(axis=-1, keepdims=True)
    return ids, w


def kernel(x, gating_output, gate_w, up_w, down_w):
    x = np.asarray(x, dtype=np.float32)
    gating_output = np.asarray(gating_output, dtype=np.float32)
    gate_w = np.asarray(gate_w, dtype=np.float32)
    up_w = np.asarray(up_w, dtype=np.float32)
    down_w = np.asarray(down_w, dtype=np.float32)

    ids, w = _route(gating_output)

    idx_e = []
    w_e = []
    for e in range(E):
        sel = np.nonzero((ids == e).any(axis=-1))[0]
        kpos = (ids[sel] == e).argmax(axis=-1)
        idx_e.append(sel)
        w_e.append(w[sel, kpos])

    cap = max(len(i) for i in idx_e)
    cap += cap & 1

    nc = _build(cap)

    in_maps = []
    for e in range(E):
        idx = idx_e[e]
        cnt = len(idx)
        x_pad = np.zeros((cap, D), dtype=np.float32)
        x_pad[:cnt] = x[idx]

        x_dev = np.ascontiguousarray(
            x_pad.T.reshape(D // P, P, cap).transpose(1, 0, 2)).astype(BF)
        gwT = gate_w[e].T
        gw_dev = np.ascontiguousarray(
            gwT.reshape(D // P, P, F // P, P).transpose(1, 2, 0, 3)).astype(BF)
        uwT = up_w[e].T
        uw_dev = np.ascontiguousarray(
            uwT.reshape(D // P, P, F // P, P).transpose(1, 2, 0, 3)).astype(BF)
        dwT = down_w[e].T
        dw_dev = np.ascontiguousarray(
            dwT.reshape(F // P, P, D).transpose(1, 0, 2)).astype(BF)

        # Pack ft0/ft1's stationary tiles after each d-slice's x columns
        # (all share the d-inner partition dim) so the ramp-critical
        # weights arrive on the full-bandwidth x stream.
        x_pack = np.empty((P, D // P, cap + 4 * P), dtype=BF)
        x_pack[:, :, :cap] = x_dev
        for ft in range(2):
            x_pack[:, :, cap + ft * 2 * P: cap + ft * 2 * P + P] = gw_dev[:, ft]
            x_pack[:, :, cap + ft * 2 * P + P: cap + (ft + 1) * 2 * P] = uw_dev[:, ft]

        in_maps.append({"x": x_pack, "gw": gw_dev, "uw": uw_dev, "dw": dw_dev})

    def _run():
        try:
            return run_bass_kernel_spmd(nc, in_maps, core_ids=list(range(N_CORES)))
        except Exception:
            import time as _time

            _time.sleep(5)
            return run_bass_kernel_spmd(nc, in_maps, core_ids=list(range(N_CORES)))

    def _assemble(res):
        out = np.zeros((T, D), dtype=np.float32)
        for e in range(E):
            cnt = len(idx_e[e])
            o = res.results[e]["out"].astype(np.float32).transpose(2, 1, 0).reshape(cap, D)
            out[idx_e[e]] += o[:cnt] * w_e[e][:, None]
        return out

    def _spot_check(out):
        worst = 0.0
        for e in range(E):
            if len(idx_e[e]) == 0:
                continue
            t = int(idx_e[e][0])
            acc = np.zeros(D, dtype=np.float32)
            for k in range(TOPK):
                ek = int(ids[t, k])
                g = gate_w[ek] @ x[t]
                u = up_w[ek] @ x[t]
                h = (g / (1.0 + np.exp(-g))) * u
                acc += w[t, k] * (down_w[ek] @ h)
            scale = np.abs(acc).max() + 1e-6
            worst = max(worst, np.abs(out[t] - acc).max() / scale)
        return worst

    res = _run()
    out = _assemble(res)
    if _spot_check(out) > 0.1:
        res = _run()
        out = _assemble(res)
    return out
